# revision 10
# baseline (speedup 1.0000x reference)
"""Trainium2 Bass kernel for nn_ASAAttention (sparse syntax-aware attention).

Sharding: 8 cores = 2 batches x 4 query-groups. Core c handles batch c//4 and
query tiles {r, 4+r, 8+r, 12+r} (r = c%4), 128 rows each -- strided so every
core runs the identical SPMD program with balanced causal work.

Per core:
  phase A: Q/K/V projections from host-transposed x^T (fp32r matmuls).
           K^T bounced through DRAM scratch (streamed back per key tile),
           V (+ones column) and Q^T kept resident in SBUF.
  phase B+C fused: for each key tile jt: build the bonding-gate strip
           G = exp(compat2) * mask once (shared across heads), then per head:
           w = exp(QK/8) * G, accumulate (w^T @ [V|1]) in PSUM over jt.
           Mask algebra (all exact 0/1 ints in fp32):
             m  = is_ge(pc + fs + D, 2)
             pc = 2*pos_mask - is_pron_i*is_noun_j   (K=18 one-hot matmul)
             fs = is_ge(compat - thr_i, 0)           (K=65 fp32 matmul)
             D  = host-baked additive tile: 0 interior, +8 eye, -8 non-causal
           Per-query factors e^{-thr_i} cancel in the softmax ratio.
  tail:    normalize by the ones-column sum, PE-transpose context,
           output projection (fp32r), + biases.
"""

import os
import sys
import numpy as np

for p in ("/opt/trn_rl_repo", "/opt/pypackages", "/root/.axon_site",
          "/root/.axon_site/_ro/trn_rl_repo", "/root/.axon_site/_ro/pypackages"):
    if os.path.isdir(p) and p not in sys.path:
        sys.path.append(p)

import concourse.bass as bass
import concourse.tile as tile
from concourse import bacc, mybir
from concourse.bass_utils import run_bass_kernel_spmd
from concourse.masks import make_identity

F32 = mybir.dt.float32
F32R = mybir.dt.float32r
AF = mybir.ActivationFunctionType
OP = mybir.AluOpType

# ---------------------------------------------------------------- constants
POS_TAGS = ['NOUN','VERB','ADJ','ADV','PRON','PROPN','DET','ADP','AUX','CCONJ',
            'SCONJ','NUM','PART','INTJ','PUNCT','SYM','X']
NUM_POS = 17
POS_TO_ID = {p: i for i, p in enumerate(POS_TAGS)}

def _build_pos_matrix():
    m = np.zeros((NUM_POS, NUM_POS), dtype=np.float32)
    pairs = [('NOUN','VERB'),('PROPN','VERB'),('PRON','VERB'),('NOUN','ADJ'),
             ('PROPN','ADJ'),('PRON','ADJ'),('VERB','VERB'),('ADJ','NOUN'),
             ('ADJ','PROPN'),('DET','NOUN'),('DET','PROPN'),('NUM','NOUN'),
             ('ADP','NOUN'),('ADP','PROPN'),('ADP','PRON'),('NOUN','NOUN'),
             ('PROPN','NOUN'),('NOUN','PROPN'),('PROPN','PROPN'),('ADV','VERB'),
             ('ADV','ADJ'),('ADV','ADV'),('AUX','VERB'),('SCONJ','VERB'),
             ('AUX','ADJ'),('AUX','NOUN'),('CCONJ','NOUN'),('CCONJ','VERB'),
             ('CCONJ','ADJ'),('CCONJ','ADV'),('CCONJ','PROPN'),('PRON','NOUN'),
             ('PRON','PROPN')]
    for dep, head in pairs:
        d, h = POS_TO_ID[dep], POS_TO_ID[head]
        m[d, h] = m[h, d] = 1.0
    for i in range(NUM_POS):
        m[i, i] = 1.0
    p = POS_TO_ID['PUNCT']
    m[p, :] = 1.0
    m[:, p] = 1.0
    return m

POS_MATRIX = _build_pos_matrix()
PRON_ID = POS_TO_ID['PRON']
NOUN_ID = POS_TO_ID['NOUN']
PROPN_ID = POS_TO_ID['PROPN']

B, S, D, H, DH, F = 2, 2048, 768, 12, 64, 64
NT = S // 128            # 16 key tiles
NCORES = 8
NQ = 4                   # query tiles per core
HP = H // 2              # 6 head pairs
SCALE = 1.0 / np.sqrt(DH)

# per key-tile jt: first query-strip block that can attend to it (exact)
KMIN = [min(NQ - 1, max(0, -(-(jt - 3) // 4))) for jt in range(NT)]
N_EXACT = [(NQ - k) * 128 for k in KMIN]                 # mask/G/w width
N_QK = [max(256, n) for n in N_EXACT]                    # fp32r wants N>=256
DOFF = np.concatenate([[0], np.cumsum(N_EXACT)]).astype(int)
DTOT = int(DOFF[-1])                                     # 5120

HEAD_PASSES = 2
HPP = H // HEAD_PASSES   # heads per pass


# ---------------------------------------------------------------- program
def build_program():
    nc = bacc.Bacc("TRN2", target_bir_lowering=False, debug=False,
                   num_devices=NCORES)

    def din(name, shape, dt=F32):
        return nc.dram_tensor(name, list(shape), dt, kind="ExternalInput").ap()

    inp = dict(
        xT=din("xT", (D, S), F32R),
        xTq=din("xTq", (D, NQ * 128), F32R),
        wqT=din("wqT", (D, D), F32R),
        wkT=din("wkT", (D, D), F32R),
        wvT=din("wvT", (D, D), F32R),
        woT=din("woT", (D, D), F32R),
        bq=din("bq", (D,)),
        bk=din("bk", (D,)),
        bv=din("bv", (D,)),
        bo=din("bo", (D,)),
        featP=din("featP", (F + 1, S)),
        reqP=din("reqP", (F + 1, NQ * 128)),
        onehotJ=din("onehotJ", (NUM_POS + 1, S), F32R),
        hostA2=din("hostA2", (NUM_POS + 1, NQ * 128), F32R),
        dstack=din("dstack", (128, DTOT)),
    )
    out = nc.dram_tensor("out", [NQ * 128, D], F32, kind="ExternalOutput").ap()

    with tile.TileContext(nc) as tc:
        _emit(tc, nc, inp, out)
    nc.compile()
    return nc


def _emit(tc, nc, inp, out):
    from contextlib import ExitStack
    ctx = ExitStack()
    with ctx:
        # ------------------------------------------------ persistent pools
        p_const = ctx.enter_context(tc.tile_pool(name="const", bufs=1))
        p_w     = ctx.enter_context(tc.tile_pool(name="wts", bufs=7))
        p_vres  = ctx.enter_context(tc.tile_pool(name="vres", bufs=1))
        p_qt    = ctx.enter_context(tc.tile_pool(name="qt", bufs=1))
        p_dram  = ctx.enter_context(tc.tile_pool(name="dram", bufs=1, space="DRAM"))
        ps_strip = ctx.enter_context(tc.tile_pool(name="pstrip", bufs=4, space="PSUM"))
        ps_acc   = ctx.enter_context(tc.tile_pool(name="pacc", bufs=1, space="PSUM"))

        # ------------------------------------------------ constants / small
        ident = p_const.tile([128, 128], F32, tag="ident", name="ident")
        make_identity(nc, ident)

        bq_sb = p_const.tile([64, H], F32, tag="bq", name="bq_sb")
        for h in range(H):
            nc.sync.dma_start(
                out=bq_sb[:, h:h+1],
                in_=inp["bq"][h*64:(h+1)*64].rearrange("(p o) -> p o", o=1))
        bk_sb = p_const.tile([128, HP], F32, tag="bk", name="bk_sb")
        for hp in range(HP):
            nc.sync.dma_start(
                out=bk_sb[:, hp:hp+1],
                in_=inp["bk"][hp*128:(hp+1)*128].rearrange("(p o) -> p o", o=1))
        bvb = p_const.tile([128, D], F32, tag="bvb", name="bvb")
        bv_ap = inp["bv"]
        nc.sync.dma_start(out=bvb, in_=bass.AP(tensor=bv_ap.tensor, offset=bv_ap.offset,
                                               ap=[[0, 128]] + list(bv_ap.ap)))
        bob = p_const.tile([128, D], F32, tag="bob", name="bob")
        bo_ap = inp["bo"]
        nc.sync.dma_start(out=bob, in_=bass.AP(tensor=bo_ap.tensor, offset=bo_ap.offset,
                                               ap=[[0, 128]] + list(bo_ap.ap)))

        # V (+ones) resident: per head [128, NT*65], s-tile-major
        vres = [p_vres.tile([128, NT * 65], F32, tag=f"v{h}", name=f"vres{h}")
                for h in range(H)]
        for h in range(H):
            nc.vector.memset(
                vres[h].rearrange("p (t c) -> p t c", c=65)[:, :, 64:65], 1.0)
        # Q^T resident: per head [64, NQ*128]
        qt_sb = [p_qt.tile([64, NQ * 128], F32R, tag=f"qt{h}", name=f"qt{h}")
                 for h in range(H)]
        # K^T DRAM bounce: [jt, d-within-pair, hp, key col]
        ktj = p_dram.tile([NT, 128, HP, 128], F32R, tag="ktj", name="ktj")

        # ------------------------------------------------ phase A: projections
        with tc.tile_pool(name="xts", bufs=1) as p_xt, \
             tc.tile_pool(name="kbp", bufs=3) as p_kb:
            xt = []
            for kt in range(6):
                t = p_xt.tile([128, S], F32R, tag=f"xt{kt}", name=f"xt{kt}")
                nc.sync.dma_start(out=t, in_=inp["xT"][kt*128:(kt+1)*128, :])
                xt.append(t)
            xtq = []
            for kt in range(6):
                t = p_xt.tile([128, NQ * 128], F32R, tag=f"xq{kt}", name=f"xtq{kt}")
                nc.sync.dma_start(out=t, in_=inp["xTq"][kt*128:(kt+1)*128, :])
                xtq.append(t)

            def wload(which, kt):
                t = p_w.tile([128, D], F32R, tag="w", name=f"w_{which}_{kt}")
                nc.sync.dma_start(out=t, in_=inp[which][kt*128:(kt+1)*128, :])
                return t

            # K projection -> DRAM bounce
            wk = [wload("wkT", kt) for kt in range(6)]
            for hp in range(HP):
                for chunk in range(4):
                    ps = ps_strip.tile([128, 512], F32, tag="strip", name="psk")
                    for kt in range(6):
                        nc.tensor.matmul(
                            ps,
                            lhsT=wk[kt][:, hp*128:(hp+1)*128],
                            rhs=xt[kt][:, chunk*512:(chunk+1)*512],
                            start=(kt == 0), stop=(kt == 5))
                    kb = p_kb.tile([128, 512], F32R, tag="kb", name="kb")
                    nc.scalar.activation(kb, ps, AF.Identity,
                                         bias=bk_sb[:, hp:hp+1], scale=1.0)
                    for jj in range(4):
                        nc.sync.dma_start(out=ktj[chunk*4+jj, :, hp, :],
                                          in_=kb[:, jj*128:(jj+1)*128])

            # Q projection (core's query columns only), per head so the
            # QK matmul operands share base partition 0
            wq = [wload("wqT", kt) for kt in range(6)]
            for h in range(H):
                psq = ps_strip.tile([64, 512], F32, tag="strip", name="psq")
                for kt in range(6):
                    nc.tensor.matmul(
                        psq,
                        lhsT=wq[kt][:, h*64:(h+1)*64],
                        rhs=xtq[kt],
                        start=(kt == 0), stop=(kt == 5))
                nc.scalar.activation(qt_sb[h], psq, AF.Identity,
                                     bias=bq_sb[:, h:h+1], scale=1.0)

            # V projection: natural [s, d] per s-tile
            wv = [wload("wvT", kt) for kt in range(6)]
            for st in range(NT):
                for half in range(2):
                    ps = ps_strip.tile([128, 384], F32, tag="strip", name="psv")
                    for kt in range(6):
                        nc.tensor.matmul(
                            ps,
                            lhsT=xt[kt][:, st*128:(st+1)*128],
                            rhs=wv[kt][:, half*384:(half+1)*384],
                            start=(kt == 0), stop=(kt == 5))
                    for hh in range(6):
                        h = half * 6 + hh
                        nc.vector.tensor_add(
                            vres[h][:, st*65:st*65+64],
                            ps[:, hh*64:(hh+1)*64],
                            bvb[:, h*64:(h+1)*64])

        # mask inputs (loaded after projections to keep phase-A SBUF low)
        p_mc = ctx.enter_context(tc.tile_pool(name="mconst", bufs=1))
        featP_sb = p_mc.tile([F + 1, S], F32, tag="featP", name="featP_sb")
        nc.sync.dma_start(out=featP_sb, in_=inp["featP"])
        reqP_sb = p_mc.tile([F + 1, NQ * 128], F32, tag="reqP", name="reqP_sb")
        nc.sync.dma_start(out=reqP_sb, in_=inp["reqP"])
        onehotJ_sb = p_mc.tile([NUM_POS + 1, S], F32R, tag="oneh", name="onehotJ_sb")
        nc.sync.dma_start(out=onehotJ_sb, in_=inp["onehotJ"])
        hostA2_sb = p_mc.tile([NUM_POS + 1, NQ * 128], F32R, tag="hA2", name="hostA2_sb")
        nc.sync.dma_start(out=hostA2_sb, in_=inp["hostA2"])

        # ------------------------------------------------ attention-phase pools
        p_kt    = ctx.enter_context(tc.tile_pool(name="kts", bufs=2))
        p_g     = ctx.enter_context(tc.tile_pool(name="gca", bufs=1))
        p_d     = ctx.enter_context(tc.tile_pool(name="dst", bufs=2))
        p_work  = ctx.enter_context(tc.tile_pool(name="wrk", bufs=2))
        p_e     = ctx.enter_context(tc.tile_pool(name="exp", bufs=2))
        p_wm    = ctx.enter_context(tc.tile_pool(name="wmul", bufs=2))
        p_ctx   = ctx.enter_context(tc.tile_pool(name="ctxT", bufs=1))
        p_norm  = ctx.enter_context(tc.tile_pool(name="nrm", bufs=3))
        p_out   = ctx.enter_context(tc.tile_pool(name="outp", bufs=2))

        # ------------------------------------------------ phases B+C
        g_cache = [None] * NT
        ctxT = [[p_ctx.tile([128, 128], F32R, tag=f"ct{k}_{hp}", name=f"ctxT{k}_{hp}")
                 for hp in range(HP)] for k in range(NQ)]

        for h in range(H):
            hp, ho = h // 2, (h % 2) * 64
            # one PSUM bank per causal q-block accumulator, reused across heads
            accs = [ps_acc.tile([128, 65], F32, tag=f"k{k}", name=f"acc{k}")
                    for k in range(NQ)]
            for jt in range(NT):
                km = KMIN[jt]
                ne = N_EXACT[jt]
                nq = N_QK[jt]
                ecols = slice(NQ*128 - ne, NQ*128)
                qcols = slice(NQ*128 - nq, NQ*128)

                if h == 0:
                    # --- bonding gate G[jt] (shared across all heads)
                    ps_c = ps_strip.tile([128, 512], F32, tag="strip", name="ps_c")
                    nc.tensor.matmul(ps_c[:, :ne],
                                     lhsT=featP_sb[:, jt*128:(jt+1)*128],
                                     rhs=reqP_sb[:, ecols],
                                     start=True, stop=True)
                    ps_p = ps_strip.tile([128, 512], F32, tag="strip", name="ps_p")
                    nc.tensor.matmul(ps_p[:, :ne],
                                     lhsT=onehotJ_sb[:, jt*128:(jt+1)*128],
                                     rhs=hostA2_sb[:, ecols],
                                     start=True, stop=True)
                    d_sb = p_d.tile([128, 512], F32, tag="d", name="d_sb")
                    nc.sync.dma_start(out=d_sb[:, :ne],
                                      in_=inp["dstack"][:, int(DOFF[jt]):int(DOFF[jt+1])])
                    fs = p_work.tile([128, 512], F32, tag="fs", name="fs")
                    nc.vector.tensor_scalar(fs[:, :ne], ps_c[:, :ne], 0.0, None,
                                            OP.is_ge)
                    nc.vector.tensor_add(fs[:, :ne], fs[:, :ne], ps_p[:, :ne])
                    nc.vector.tensor_add(fs[:, :ne], fs[:, :ne], d_sb[:, :ne])
                    msk = p_work.tile([128, 512], F32, tag="msk", name="msk")
                    nc.vector.tensor_scalar(msk[:, :ne], fs[:, :ne], 2.0, None,
                                            OP.is_ge)
                    ec = p_e.tile([128, 512], F32, tag="ec", name="ec", bufs=2)
                    nc.scalar.activation(ec[:, :ne], ps_c[:, :ne], AF.Exp)
                    g = p_g.tile([128, ne], F32, tag=f"g{jt}", name=f"g{jt}")
                    nc.vector.tensor_mul(g, ec[:, :ne], msk[:, :ne])
                    g_cache[jt] = g

                g = g_cache[jt]
                kth = p_kt.tile([64, 128], F32R, tag="kt", name="kth")
                nc.sync.dma_start(out=kth, in_=ktj[jt, ho:ho+64, hp, :])

                ps_qk = ps_strip.tile([128, 512], F32, tag="strip", name="ps_qk")
                nc.tensor.matmul(
                    ps_qk[:, :nq],
                    lhsT=kth,
                    rhs=qt_sb[h][:, qcols],
                    start=True, stop=True)
                e = p_e.tile([128, 512], F32, tag="e", name="e")
                nc.scalar.activation(e[:, :ne], ps_qk[:, nq-ne:nq], AF.Exp,
                                     scale=float(SCALE))
                w = p_wm.tile([128, 512], F32, tag="w", name="w")
                nc.vector.tensor_mul(w[:, :ne], e[:, :ne], g)

                for k in range(km, NQ):
                    nc.tensor.matmul(
                        accs[k],
                        lhsT=w[:, (k-km)*128:(k-km+1)*128],
                        rhs=vres[h][:, jt*65:(jt+1)*65],
                        start=(jt == 0), stop=(jt == 4*k + 3))
                    if jt == 4*k + 3:
                        r = p_norm.tile([128, 1], F32, tag="r", name="rcp")
                        nc.vector.reciprocal(r, accs[k][:, 64:65])
                        cs = p_norm.tile([128, 64], F32, tag="cs", name="cs")
                        nc.vector.tensor_scalar(cs, accs[k][:, 0:64],
                                                r, None, OP.mult)
                        ps_t = ps_strip.tile([64, 128], F32, tag="strip",
                                             name="ps_t")
                        nc.tensor.transpose(ps_t, cs, ident)
                        if ho == 0:
                            nc.vector.tensor_copy(ctxT[k][hp][0:64, :], ps_t)
                        else:
                            cs2 = p_norm.tile([64, 128], F32R, tag="cs2", name="cs2")
                            nc.vector.tensor_copy(cs2, ps_t)
                            nc.sync.dma_start(out=ctxT[k][hp][64:128, :], in_=cs2)

        # ------------------------------------------------ tail: out projection
        wo = []
        for kt in range(6):
            t = p_w.tile([128, D], F32R, tag="w", name=f"w_wo_{kt}")
            nc.sync.dma_start(out=t, in_=inp["woT"][kt*128:(kt+1)*128, :])
            wo.append(t)
        for k in range(NQ):
            for half in range(2):
                ps_o = ps_strip.tile([128, 384], F32, tag="strip", name="ps_o")
                for m in range(6):
                    nc.tensor.matmul(
                        ps_o,
                        lhsT=ctxT[k][m],
                        rhs=wo[m][:, half*384:(half+1)*384],
                        start=(m == 0), stop=(m == 5))
                ob = p_out.tile([128, 384], F32, tag="ob", name="ob")
                nc.vector.tensor_add(ob, ps_o, bob[:, half*384:(half+1)*384])
                nc.sync.dma_start(out=out[k*128:(k+1)*128, half*384:(half+1)*384],
                                  in_=ob)


# ---------------------------------------------------------------- host side
_NC_CACHE = None

def _get_program():
    global _NC_CACHE
    if _NC_CACHE is None:
        _NC_CACHE = build_program()
    return _NC_CACHE


def core_rows(c):
    r = c % 4
    return np.concatenate([np.arange((4*k + r)*128, (4*k + r + 1)*128)
                           for k in range(NQ)])


def prep_in_maps(x, features, requirements, pos_ids,
                 W_q, b_q, W_k, b_k, W_v, b_v, W_o, b_o):
    x = np.asarray(x, np.float32)
    features = np.asarray(features, np.float32)
    requirements = np.asarray(requirements, np.float32)
    pos_ids = np.asarray(pos_ids)

    shared = []
    for b in range(B):
        featP = np.empty((F + 1, S), np.float32)
        featP[:F] = features[b].T
        featP[F] = 1.0
        onehotJ = np.zeros((NUM_POS + 1, S), np.float32)
        for t in range(NUM_POS):
            onehotJ[t] = (pos_ids[b] == t)
        onehotJ[NUM_POS] = ((pos_ids[b] == NOUN_ID) | (pos_ids[b] == PROPN_ID))
        shared.append(dict(
            xT=np.ascontiguousarray(x[b].T),
            wqT=np.ascontiguousarray(np.asarray(W_q, np.float32).T),
            wkT=np.ascontiguousarray(np.asarray(W_k, np.float32).T),
            wvT=np.ascontiguousarray(np.asarray(W_v, np.float32).T),
            woT=np.ascontiguousarray(np.asarray(W_o, np.float32).T),
            bq=np.asarray(b_q, np.float32), bk=np.asarray(b_k, np.float32),
            bv=np.asarray(b_v, np.float32), bo=np.asarray(b_o, np.float32),
            featP=featP, onehotJ=onehotJ,
        ))

    tri = np.tril(np.full((128, 128), -8.0, np.float32), -1)  # jp > if -> -8
    np.fill_diagonal(tri, 8.0)                                # eye -> +8

    in_maps, rows_l = [], []
    for c in range(NCORES):
        b, r = c // 4, c % 4
        rows = core_rows(c)

        req_rows = requirements[b][rows]
        rc = req_rows.sum(-1)
        inv = 1.0 / (rc + 1e-6)
        thr = rc * inv

        reqP = np.empty((F + 1, NQ * 128), np.float32)
        reqP[:F] = (req_rows * inv[:, None]).T
        reqP[F] = -thr

        pos_core = pos_ids[b][rows]
        hostA2 = np.empty((NUM_POS + 1, NQ * 128), np.float32)
        hostA2[:NUM_POS] = 2.0 * POS_MATRIX[pos_core].T
        hostA2[NUM_POS] = -(pos_core == PRON_ID).astype(np.float32)

        dstack = np.zeros((128, DTOT), np.float32)
        for jt in range(NT):
            for k in range(KMIN[jt], NQ):
                it = 4*k + r
                blk = dstack[:, int(DOFF[jt]) + (k - KMIN[jt])*128:
                             int(DOFF[jt]) + (k - KMIN[jt] + 1)*128]
                if jt == it:
                    blk[:] = tri
                elif jt > it:
                    blk[:] = -8.0

        m = dict(shared[b])
        m["xTq"] = np.ascontiguousarray(shared[b]["xT"][:, rows])
        m["reqP"] = reqP
        m["hostA2"] = hostA2
        m["dstack"] = dstack
        in_maps.append(m)
        rows_l.append(rows)
    return in_maps, rows_l


def run(inputs, trace=False):
    in_maps, rows_l = prep_in_maps(**inputs)
    nc = _get_program()
    res = run_bass_kernel_spmd(nc, in_maps, core_ids=list(range(NCORES)),
                               trace=trace)
    outf = np.empty((B, S, D), np.float32)
    for c in range(NCORES):
        outf[c // 4, rows_l[c]] = res.results[c]["out"]
    return outf, res


def kernel(**inputs):
    outf, _ = run(inputs, trace=False)
    return outf


# revision 11
# speedup vs baseline: 1.1005x; 1.1005x over previous
"""Trainium2 Bass kernel for nn_ASAAttention (sparse syntax-aware attention).

Sharding: 8 cores = 2 batches x 4 query-groups. Core c handles batch c//4 and
query tiles {r, 4+r, 8+r, 12+r} (r = c%4), 128 rows each -- strided so every
core runs the identical SPMD program with balanced causal work.

Per core:
  phase A: Q/K/V projections from host-transposed x^T (fp32r matmuls).
           K^T bounced through DRAM scratch (streamed back per key tile),
           V (+ones column) and Q^T kept resident in SBUF.
  phase B+C fused: for each key tile jt: build the bonding-gate strip
           G = exp(compat2) * mask once (shared across heads), then per head:
           w = exp(QK/8) * G, accumulate (w^T @ [V|1]) in PSUM over jt.
           Mask algebra (all exact 0/1 ints in fp32):
             m  = is_ge(pc + fs + D, 2)
             pc = 2*pos_mask - is_pron_i*is_noun_j   (K=18 one-hot matmul)
             fs = is_ge(compat - thr_i, 0)           (K=65 fp32 matmul)
             D  = host-baked additive tile: 0 interior, +8 eye, -8 non-causal
           Per-query factors e^{-thr_i} cancel in the softmax ratio.
  tail:    normalize by the ones-column sum, PE-transpose context,
           output projection (fp32r), + biases.
"""

import os
import sys
import numpy as np

for p in ("/opt/trn_rl_repo", "/opt/pypackages", "/root/.axon_site",
          "/root/.axon_site/_ro/trn_rl_repo", "/root/.axon_site/_ro/pypackages"):
    if os.path.isdir(p) and p not in sys.path:
        sys.path.append(p)

import concourse.bass as bass
import concourse.tile as tile
from concourse import bacc, mybir
from concourse.bass_utils import run_bass_kernel_spmd
from concourse.masks import make_identity

F32 = mybir.dt.float32
F32R = mybir.dt.float32r
AF = mybir.ActivationFunctionType
OP = mybir.AluOpType

# ---------------------------------------------------------------- constants
POS_TAGS = ['NOUN','VERB','ADJ','ADV','PRON','PROPN','DET','ADP','AUX','CCONJ',
            'SCONJ','NUM','PART','INTJ','PUNCT','SYM','X']
NUM_POS = 17
POS_TO_ID = {p: i for i, p in enumerate(POS_TAGS)}

def _build_pos_matrix():
    m = np.zeros((NUM_POS, NUM_POS), dtype=np.float32)
    pairs = [('NOUN','VERB'),('PROPN','VERB'),('PRON','VERB'),('NOUN','ADJ'),
             ('PROPN','ADJ'),('PRON','ADJ'),('VERB','VERB'),('ADJ','NOUN'),
             ('ADJ','PROPN'),('DET','NOUN'),('DET','PROPN'),('NUM','NOUN'),
             ('ADP','NOUN'),('ADP','PROPN'),('ADP','PRON'),('NOUN','NOUN'),
             ('PROPN','NOUN'),('NOUN','PROPN'),('PROPN','PROPN'),('ADV','VERB'),
             ('ADV','ADJ'),('ADV','ADV'),('AUX','VERB'),('SCONJ','VERB'),
             ('AUX','ADJ'),('AUX','NOUN'),('CCONJ','NOUN'),('CCONJ','VERB'),
             ('CCONJ','ADJ'),('CCONJ','ADV'),('CCONJ','PROPN'),('PRON','NOUN'),
             ('PRON','PROPN')]
    for dep, head in pairs:
        d, h = POS_TO_ID[dep], POS_TO_ID[head]
        m[d, h] = m[h, d] = 1.0
    for i in range(NUM_POS):
        m[i, i] = 1.0
    p = POS_TO_ID['PUNCT']
    m[p, :] = 1.0
    m[:, p] = 1.0
    return m

POS_MATRIX = _build_pos_matrix()
PRON_ID = POS_TO_ID['PRON']
NOUN_ID = POS_TO_ID['NOUN']
PROPN_ID = POS_TO_ID['PROPN']

B, S, D, H, DH, F = 2, 2048, 768, 12, 64, 64
NT = S // 128            # 16 key tiles
NCORES = 8
NQ = 4                   # query tiles per core
HP = H // 2              # 6 head pairs
SCALE = 1.0 / np.sqrt(DH)

# per key-tile jt: first query-strip block that can attend to it (exact)
KMIN = [min(NQ - 1, max(0, -(-(jt - 3) // 4))) for jt in range(NT)]
N_EXACT = [(NQ - k) * 128 for k in KMIN]                 # mask/G/w width
N_QK = [max(256, n) for n in N_EXACT]                    # fp32r wants N>=256
DOFF = np.concatenate([[0], np.cumsum(N_EXACT)]).astype(int)
DTOT = int(DOFF[-1])                                     # 5120

HEAD_PASSES = 2
HPP = H // HEAD_PASSES   # heads per pass


# ---------------------------------------------------------------- program
def build_program():
    nc = bacc.Bacc("TRN2", target_bir_lowering=False, debug=False,
                   num_devices=NCORES)

    def din(name, shape, dt=F32):
        return nc.dram_tensor(name, list(shape), dt, kind="ExternalInput").ap()

    inp = dict(
        xT=din("xT", (D, S), F32R),
        xTq=din("xTq", (D, NQ * 128), F32R),
        wqT=din("wqT", (D, D), F32R),
        wkT=din("wkT", (D, D), F32R),
        wvT=din("wvT", (D, D), F32R),
        woT=din("woT", (D, D), F32R),
        bq=din("bq", (D,)),
        bk=din("bk", (D,)),
        bv=din("bv", (D,)),
        bo=din("bo", (D,)),
        featP=din("featP", (F + 1, S)),
        reqP=din("reqP", (F + 1, NQ * 128)),
        onehotJ=din("onehotJ", (NUM_POS + 1, S), F32R),
        hostA2=din("hostA2", (NUM_POS + 1, NQ * 128), F32R),
        dstack=din("dstack", (128, DTOT)),
    )
    out = nc.dram_tensor("out", [NQ * 128, D], F32, kind="ExternalOutput").ap()

    with tile.TileContext(nc) as tc:
        _emit(tc, nc, inp, out)
    nc.compile()
    return nc


def _emit(tc, nc, inp, out):
    from contextlib import ExitStack
    ctx = ExitStack()
    with ctx:
        # ------------------------------------------------ persistent pools
        p_const = ctx.enter_context(tc.tile_pool(name="const", bufs=1))
        p_w     = ctx.enter_context(tc.tile_pool(name="wts", bufs=7))
        p_vres  = ctx.enter_context(tc.tile_pool(name="vres", bufs=1))
        p_qt    = ctx.enter_context(tc.tile_pool(name="qt", bufs=1))
        p_dram  = ctx.enter_context(tc.tile_pool(name="dram", bufs=1, space="DRAM"))
        ps_strip = ctx.enter_context(tc.tile_pool(name="pstrip", bufs=3, space="PSUM"))
        ps_acc   = ctx.enter_context(tc.tile_pool(name="pacc", bufs=1, space="PSUM"))

        # ------------------------------------------------ constants / small
        ident = p_const.tile([128, 128], F32, tag="ident", name="ident")
        make_identity(nc, ident)

        bq_sb = p_const.tile([64, H], F32, tag="bq", name="bq_sb")
        for h in range(H):
            nc.sync.dma_start(
                out=bq_sb[:, h:h+1],
                in_=inp["bq"][h*64:(h+1)*64].rearrange("(p o) -> p o", o=1))
        bk_sb = p_const.tile([128, HP], F32, tag="bk", name="bk_sb")
        for hp in range(HP):
            nc.sync.dma_start(
                out=bk_sb[:, hp:hp+1],
                in_=inp["bk"][hp*128:(hp+1)*128].rearrange("(p o) -> p o", o=1))
        bvb = p_const.tile([128, D], F32, tag="bvb", name="bvb")
        bv_ap = inp["bv"]
        nc.sync.dma_start(out=bvb, in_=bass.AP(tensor=bv_ap.tensor, offset=bv_ap.offset,
                                               ap=[[0, 128]] + list(bv_ap.ap)))
        bob = p_const.tile([128, D], F32, tag="bob", name="bob")
        bo_ap = inp["bo"]
        nc.sync.dma_start(out=bob, in_=bass.AP(tensor=bo_ap.tensor, offset=bo_ap.offset,
                                               ap=[[0, 128]] + list(bo_ap.ap)))

        # V (+ones) resident: per head [128, NT*65], s-tile-major
        vres = [p_vres.tile([128, NT * 65], F32, tag=f"v{h}", name=f"vres{h}")
                for h in range(H)]
        for h in range(H):
            nc.vector.memset(
                vres[h].rearrange("p (t c) -> p t c", c=65)[:, :, 64:65], 1.0)
        # Q^T resident: per head [64, NQ*128]
        qt_sb = [p_qt.tile([64, NQ * 128], F32R, tag=f"qt{h}", name=f"qt{h}")
                 for h in range(H)]
        # K^T DRAM bounce: [jt, d-within-pair, hp, key col]
        ktj = p_dram.tile([NT, 128, HP, 128], F32R, tag="ktj", name="ktj")

        # ------------------------------------------------ phase A: projections
        with tc.tile_pool(name="xts", bufs=1) as p_xt, \
             tc.tile_pool(name="kbp", bufs=3) as p_kb:
            xt = []
            for kt in range(6):
                t = p_xt.tile([128, S], F32R, tag=f"xt{kt}", name=f"xt{kt}")
                nc.sync.dma_start(out=t, in_=inp["xT"][kt*128:(kt+1)*128, :])
                xt.append(t)
            xtq = []
            for kt in range(6):
                t = p_xt.tile([128, NQ * 128], F32R, tag=f"xq{kt}", name=f"xtq{kt}")
                nc.sync.dma_start(out=t, in_=inp["xTq"][kt*128:(kt+1)*128, :])
                xtq.append(t)

            def wload(which, kt):
                t = p_w.tile([128, D], F32R, tag="w", name=f"w_{which}_{kt}")
                nc.sync.dma_start(out=t, in_=inp[which][kt*128:(kt+1)*128, :])
                return t

            # K projection -> DRAM bounce
            wk = [wload("wkT", kt) for kt in range(6)]
            for hp in range(HP):
                for chunk in range(4):
                    ps = ps_strip.tile([128, 512], F32, tag="strip", name="psk")
                    for kt in range(6):
                        nc.tensor.matmul(
                            ps,
                            lhsT=wk[kt][:, hp*128:(hp+1)*128],
                            rhs=xt[kt][:, chunk*512:(chunk+1)*512],
                            start=(kt == 0), stop=(kt == 5))
                    kb = p_kb.tile([128, 512], F32R, tag="kb", name="kb")
                    nc.scalar.activation(kb, ps, AF.Identity,
                                         bias=bk_sb[:, hp:hp+1], scale=1.0)
                    for jj in range(4):
                        nc.sync.dma_start(out=ktj[chunk*4+jj, :, hp, :],
                                          in_=kb[:, jj*128:(jj+1)*128])

            # Q projection (core's query columns only), per head so the
            # QK matmul operands share base partition 0
            wq = [wload("wqT", kt) for kt in range(6)]
            for h in range(H):
                psq = ps_strip.tile([64, 512], F32, tag="strip", name="psq")
                for kt in range(6):
                    nc.tensor.matmul(
                        psq,
                        lhsT=wq[kt][:, h*64:(h+1)*64],
                        rhs=xtq[kt],
                        start=(kt == 0), stop=(kt == 5))
                nc.scalar.activation(qt_sb[h], psq, AF.Identity,
                                     bias=bq_sb[:, h:h+1], scale=1.0)

            # V projection: natural [s, d] per s-tile
            wv = [wload("wvT", kt) for kt in range(6)]
            for st in range(NT):
                for half in range(2):
                    ps = ps_strip.tile([128, 384], F32, tag="strip", name="psv")
                    for kt in range(6):
                        nc.tensor.matmul(
                            ps,
                            lhsT=xt[kt][:, st*128:(st+1)*128],
                            rhs=wv[kt][:, half*384:(half+1)*384],
                            start=(kt == 0), stop=(kt == 5))
                    for hh in range(6):
                        h = half * 6 + hh
                        nc.vector.tensor_add(
                            vres[h][:, st*65:st*65+64],
                            ps[:, hh*64:(hh+1)*64],
                            bvb[:, h*64:(h+1)*64])

        # mask inputs (loaded after projections to keep phase-A SBUF low)
        p_mc = ctx.enter_context(tc.tile_pool(name="mconst", bufs=1))
        featP_sb = p_mc.tile([F + 1, S], F32, tag="featP", name="featP_sb")
        nc.sync.dma_start(out=featP_sb, in_=inp["featP"])
        reqP_sb = p_mc.tile([F + 1, NQ * 128], F32, tag="reqP", name="reqP_sb")
        nc.sync.dma_start(out=reqP_sb, in_=inp["reqP"])
        onehotJ_sb = p_mc.tile([NUM_POS + 1, S], F32R, tag="oneh", name="onehotJ_sb")
        nc.sync.dma_start(out=onehotJ_sb, in_=inp["onehotJ"])
        hostA2_sb = p_mc.tile([NUM_POS + 1, NQ * 128], F32R, tag="hA2", name="hostA2_sb")
        nc.sync.dma_start(out=hostA2_sb, in_=inp["hostA2"])

        # ------------------------------------------------ attention-phase pools
        p_kt    = ctx.enter_context(tc.tile_pool(name="kts", bufs=10))
        p_g     = ctx.enter_context(tc.tile_pool(name="gca", bufs=1))
        p_d     = ctx.enter_context(tc.tile_pool(name="dst", bufs=4))
        p_work  = ctx.enter_context(tc.tile_pool(name="wrk", bufs=2))
        p_e     = ctx.enter_context(tc.tile_pool(name="exp", bufs=2))
        p_wm    = ctx.enter_context(tc.tile_pool(name="wmul", bufs=2))
        p_ctx   = ctx.enter_context(tc.tile_pool(name="ctxT", bufs=1))
        p_norm  = ctx.enter_context(tc.tile_pool(name="nrm", bufs=3))
        p_out   = ctx.enter_context(tc.tile_pool(name="outp", bufs=2))

        # ------------------------------------------------ phases B+C
        g_cache = [None] * NT
        ctxT = [[p_ctx.tile([128, 128], F32R, tag=f"ct{k}_{hp}", name=f"ctxT{k}_{hp}")
                 for hp in range(HP)] for k in range(NQ)]

        for h in range(H):
            hp, ho = h // 2, (h % 2) * 64
            # one PSUM bank per causal q-block accumulator, reused across heads
            accs = [ps_acc.tile([128, 65], F32, tag=f"k{k}", name=f"acc{k}",
                                bufs=(2 if k == NQ - 1 else 1))
                    for k in range(NQ)]
            for jt in range(NT):
                km = KMIN[jt]
                ne = N_EXACT[jt]
                nq = N_QK[jt]
                ecols = slice(NQ*128 - ne, NQ*128)
                qcols = slice(NQ*128 - nq, NQ*128)

                if h == 0:
                    # --- bonding gate G[jt] (shared across all heads)
                    ps_c = ps_strip.tile([128, 512], F32, tag="strip", name="ps_c")
                    nc.tensor.matmul(ps_c[:, :ne],
                                     lhsT=featP_sb[:, jt*128:(jt+1)*128],
                                     rhs=reqP_sb[:, ecols],
                                     start=True, stop=True)
                    ps_p = ps_strip.tile([128, 512], F32, tag="strip", name="ps_p")
                    nc.tensor.matmul(ps_p[:, :ne],
                                     lhsT=onehotJ_sb[:, jt*128:(jt+1)*128],
                                     rhs=hostA2_sb[:, ecols],
                                     start=True, stop=True)
                    d_sb = p_d.tile([128, 512], F32, tag="d", name="d_sb")
                    nc.sync.dma_start(out=d_sb[:, :ne],
                                      in_=inp["dstack"][:, int(DOFF[jt]):int(DOFF[jt+1])])
                    fs = p_work.tile([128, 512], F32, tag="fs", name="fs")
                    nc.vector.tensor_scalar(fs[:, :ne], ps_c[:, :ne], 0.0, None,
                                            OP.is_ge)
                    nc.vector.tensor_add(fs[:, :ne], fs[:, :ne], ps_p[:, :ne])
                    nc.vector.tensor_add(fs[:, :ne], fs[:, :ne], d_sb[:, :ne])
                    msk = p_work.tile([128, 512], F32, tag="msk", name="msk")
                    nc.vector.tensor_scalar(msk[:, :ne], fs[:, :ne], 2.0, None,
                                            OP.is_ge)
                    ec = p_e.tile([128, 512], F32, tag="ec", name="ec", bufs=2)
                    nc.scalar.activation(ec[:, :ne], ps_c[:, :ne], AF.Exp)
                    g = p_g.tile([128, ne], F32, tag=f"g{jt}", name=f"g{jt}")
                    nc.vector.tensor_mul(g, ec[:, :ne], msk[:, :ne])
                    g_cache[jt] = g

                g = g_cache[jt]
                kth = p_kt.tile([64, 128], F32R, tag="kt", name="kth")
                nc.sync.dma_start(out=kth, in_=ktj[jt, ho:ho+64, hp, :])

                ps_qk = ps_strip.tile([128, 512], F32, tag="strip", name="ps_qk")
                nc.tensor.matmul(
                    ps_qk[:, :nq],
                    lhsT=kth,
                    rhs=qt_sb[h][:, qcols],
                    start=True, stop=True)
                e = p_e.tile([128, 512], F32, tag="e", name="e")
                nc.scalar.activation(e[:, :ne], ps_qk[:, nq-ne:nq], AF.Exp,
                                     scale=float(SCALE))
                w = p_wm.tile([128, 512], F32, tag="w", name="w")
                nc.vector.tensor_mul(w[:, :ne], e[:, :ne], g)

                for k in range(km, NQ):
                    nc.tensor.matmul(
                        accs[k],
                        lhsT=w[:, (k-km)*128:(k-km+1)*128],
                        rhs=vres[h][:, jt*65:(jt+1)*65],
                        start=(jt == 0), stop=(jt == 4*k + 3))
                    if jt == 4*k + 3:
                        r = p_norm.tile([128, 1], F32, tag="r", name="rcp")
                        nc.vector.reciprocal(r, accs[k][:, 64:65])
                        cs = p_norm.tile([128, 64], F32, tag="cs", name="cs")
                        nc.vector.tensor_scalar(cs, accs[k][:, 0:64],
                                                r, None, OP.mult)
                        ps_t = ps_strip.tile([64, 128], F32, tag="strip",
                                             name="ps_t")
                        nc.tensor.transpose(ps_t, cs, ident)
                        if ho == 0:
                            nc.vector.tensor_copy(ctxT[k][hp][0:64, :], ps_t)
                        else:
                            cs2 = p_norm.tile([64, 128], F32R, tag="cs2", name="cs2")
                            nc.vector.tensor_copy(cs2, ps_t)
                            nc.sync.dma_start(out=ctxT[k][hp][64:128, :], in_=cs2)

        # ------------------------------------------------ tail: out projection
        wo = []
        for kt in range(6):
            t = p_w.tile([128, D], F32R, tag="w", name=f"w_wo_{kt}")
            nc.sync.dma_start(out=t, in_=inp["woT"][kt*128:(kt+1)*128, :])
            wo.append(t)
        for k in range(NQ):
            for half in range(2):
                ps_o = ps_strip.tile([128, 384], F32, tag="strip", name="ps_o")
                for m in range(6):
                    nc.tensor.matmul(
                        ps_o,
                        lhsT=ctxT[k][m],
                        rhs=wo[m][:, half*384:(half+1)*384],
                        start=(m == 0), stop=(m == 5))
                ob = p_out.tile([128, 384], F32, tag="ob", name="ob")
                nc.vector.tensor_add(ob, ps_o, bob[:, half*384:(half+1)*384])
                nc.sync.dma_start(out=out[k*128:(k+1)*128, half*384:(half+1)*384],
                                  in_=ob)


# ---------------------------------------------------------------- host side
_NC_CACHE = None

def _get_program():
    global _NC_CACHE
    if _NC_CACHE is None:
        _NC_CACHE = build_program()
    return _NC_CACHE


def core_rows(c):
    r = c % 4
    return np.concatenate([np.arange((4*k + r)*128, (4*k + r + 1)*128)
                           for k in range(NQ)])


def prep_in_maps(x, features, requirements, pos_ids,
                 W_q, b_q, W_k, b_k, W_v, b_v, W_o, b_o):
    x = np.asarray(x, np.float32)
    features = np.asarray(features, np.float32)
    requirements = np.asarray(requirements, np.float32)
    pos_ids = np.asarray(pos_ids)

    shared = []
    for b in range(B):
        featP = np.empty((F + 1, S), np.float32)
        featP[:F] = features[b].T
        featP[F] = 1.0
        onehotJ = np.zeros((NUM_POS + 1, S), np.float32)
        for t in range(NUM_POS):
            onehotJ[t] = (pos_ids[b] == t)
        onehotJ[NUM_POS] = ((pos_ids[b] == NOUN_ID) | (pos_ids[b] == PROPN_ID))
        shared.append(dict(
            xT=np.ascontiguousarray(x[b].T),
            wqT=np.ascontiguousarray(np.asarray(W_q, np.float32).T),
            wkT=np.ascontiguousarray(np.asarray(W_k, np.float32).T),
            wvT=np.ascontiguousarray(np.asarray(W_v, np.float32).T),
            woT=np.ascontiguousarray(np.asarray(W_o, np.float32).T),
            bq=np.asarray(b_q, np.float32), bk=np.asarray(b_k, np.float32),
            bv=np.asarray(b_v, np.float32), bo=np.asarray(b_o, np.float32),
            featP=featP, onehotJ=onehotJ,
        ))

    tri = np.tril(np.full((128, 128), -8.0, np.float32), -1)  # jp > if -> -8
    np.fill_diagonal(tri, 8.0)                                # eye -> +8

    in_maps, rows_l = [], []
    for c in range(NCORES):
        b, r = c // 4, c % 4
        rows = core_rows(c)

        req_rows = requirements[b][rows]
        rc = req_rows.sum(-1)
        inv = 1.0 / (rc + 1e-6)
        thr = rc * inv

        reqP = np.empty((F + 1, NQ * 128), np.float32)
        reqP[:F] = (req_rows * inv[:, None]).T
        reqP[F] = -thr

        pos_core = pos_ids[b][rows]
        hostA2 = np.empty((NUM_POS + 1, NQ * 128), np.float32)
        hostA2[:NUM_POS] = 2.0 * POS_MATRIX[pos_core].T
        hostA2[NUM_POS] = -(pos_core == PRON_ID).astype(np.float32)

        dstack = np.zeros((128, DTOT), np.float32)
        for jt in range(NT):
            for k in range(KMIN[jt], NQ):
                it = 4*k + r
                blk = dstack[:, int(DOFF[jt]) + (k - KMIN[jt])*128:
                             int(DOFF[jt]) + (k - KMIN[jt] + 1)*128]
                if jt == it:
                    blk[:] = tri
                elif jt > it:
                    blk[:] = -8.0

        m = dict(shared[b])
        m["xTq"] = np.ascontiguousarray(shared[b]["xT"][:, rows])
        m["reqP"] = reqP
        m["hostA2"] = hostA2
        m["dstack"] = dstack
        in_maps.append(m)
        rows_l.append(rows)
    return in_maps, rows_l


def run(inputs, trace=False):
    in_maps, rows_l = prep_in_maps(**inputs)
    nc = _get_program()
    res = run_bass_kernel_spmd(nc, in_maps, core_ids=list(range(NCORES)),
                               trace=trace)
    outf = np.empty((B, S, D), np.float32)
    for c in range(NCORES):
        outf[c // 4, rows_l[c]] = res.results[c]["out"]
    return outf, res


def kernel(**inputs):
    outf, _ = run(inputs, trace=False)
    return outf
